# revision 6
# baseline (speedup 1.0000x reference)
"""Trainium2 Bass kernel for CARC attention processor.

Full computation:
    q/k/v = split_heads(hidden @ W{q,k,v})
    k_full = concat([k, ALPHA*K_bg], seq); v_full likewise
    scores = q @ k_full^T * scale + mask (mask zero over bg segment)
    out = softmax(scores) @ v_full  -> merge heads -> @ Wo + bo

Sharding: data-parallel over the B*H = 16 batched heads; core c owns the
adjacent head pair bh = (2c, 2c+1), both from batch b = c//4.  Projection
weight column/row slices for the pair are shipped per core; each core
returns its partial output contribution (its two heads through Wo) and the
host sums the four partials per batch and adds the bias.

Device-side layout: scores are computed transposed ([kv_chunk=128, q]) so
that the softmax denominator falls out of the PV matmul via a ones-column
appended to V (PE reduces over partitions), and probs feed the PV matmul
with no transposes anywhere.  The additive mask is applied multiplicatively
AFTER exp -- the host ships exp(mask) and the device computes
P = exp(S*scale) * expmask.  This keeps the scalar engine (which can read
PSUM) as the sole consumer of score banks on a fixed cadence for both
heads, so the two heads' K=64 score matmuls stay adjacent in the PE queue
and run concurrently on row-tiles (0,0)/(64,0); the DVE only does a cheap
all-16-bit elementwise multiply per self chunk.  Exp of the bg segment
folds alpha*scale into the activation scale immediate.

All inputs are pre-converted to bf16 on the host (matmuls run in bf16
anyway; fp32 is double-pass half-rate on trn2 PE), halving HBM traffic and
eliminating every on-device fp32->bf16 conversion pass.  V_bg is
pre-scaled by ALPHA on the host.  The two heads' score matmuls are K=64
and issued back-to-back on row-tiles (0,0)/(64,0) so the PE runs them
concurrently.  The masked scores stay fp32 into exp, accumulation is fp32
in PSUM, and the softmax normalization (reciprocal + broadcast) is exact
fp32.
"""

import math

import numpy as np
import ml_dtypes

import concourse.bass as bass  # noqa: F401
import concourse.tile as tile
from concourse import bacc, mybir
from concourse.bass_utils import run_bass_kernel_spmd

F32 = mybir.dt.float32
BF16 = mybir.dt.bfloat16
FP16 = mybir.dt.float16
BF16_NP = ml_dtypes.bfloat16

B, H, LQ, LBG, DH = 2, 8, 2048, 2048, 64
C = H * DH  # 512
ALPHA = 0.48
SCALE = 1.0 / math.sqrt(DH)
N_CORES = 8
HPC = 2  # heads per core

VE = DH + 1  # v tile width incl. ones column


def build_program(lq=LQ, lbg=LBG, c=C, nq=None):
    """Per-core program. All cores run the same NEFF on different data."""
    nq = nq or min(1024, lq)
    assert lq % 128 == 0 and lbg % 128 == 0 and c % 128 == 0 and lq % nq == 0
    n_qh = lq // nq  # q column blocks
    n_cc = c // 128  # contraction chunks for projections
    n_ts = lq // 128  # self kv / t tiles
    n_tb = lbg // 128  # bg kv tiles
    n_j = n_ts + n_tb  # kv chunks per head
    nw = min(nq, 512)  # matmul N slice (one PSUM bank)
    ncol = min(c, 512)

    nc = bacc.Bacc("TRN2", target_bir_lowering=False, debug=False)

    hT = nc.dram_tensor("hT", [c, lq], BF16, kind="ExternalInput")
    maskT = nc.dram_tensor("maskT", [lq, lq], BF16, kind="ExternalInput")
    kbgT = nc.dram_tensor("kbgT", [HPC * DH, lbg], BF16, kind="ExternalInput")
    vbg = nc.dram_tensor("vbg", [HPC, lbg, DH], BF16, kind="ExternalInput")
    wq2 = nc.dram_tensor("wq2", [c, HPC * DH], BF16, kind="ExternalInput")
    wk2 = nc.dram_tensor("wk2", [c, HPC * DH], BF16, kind="ExternalInput")
    wv2 = nc.dram_tensor("wv2", [c, HPC * DH], BF16, kind="ExternalInput")
    wo2 = nc.dram_tensor("wo2", [HPC * DH, c], BF16, kind="ExternalInput")
    outp = nc.dram_tensor("outp", [lq, c], F32, kind="ExternalOutput")

    with tile.TileContext(nc) as tc:
        with (
            tc.tile_pool(name="persist", bufs=1) as persist,
            tc.tile_pool(name="att_sb", bufs=3) as ab,
            tc.tile_pool(name="m_sb", bufs=min(12, n_ts)) as mb,
            tc.tile_pool(name="dram_p", bufs=2, space="DRAM") as dp,
        ):
            qT = persist.tile([128, lq], BF16)  # rows 0:64 head0, 64:128 head1
            kT = persist.tile([128, lq], BF16)
            kbgT_sb = persist.tile([128, lbg], BF16)
            vself = [
                persist.tile([128, n_ts * VE], BF16, name=f"vself{h}")
                for h in range(HPC)
            ]
            vbg_sb = [
                persist.tile([128, n_tb * VE], BF16, name=f"vbgsb{h}")
                for h in range(HPC)
            ]
            ctx2 = persist.tile([128, lq], BF16)  # rows: [h0 d | h1 d], cols: q
            ctxr = persist.tile([128, lq], BF16)  # unnormalized ctx
            dens = [
                persist.tile([1, lq], F32, name=f"den{h}") for h in range(HPC)
            ]  # softmax denominators
            wo_sb = persist.tile([HPC * DH, c], BF16)

            mask_tiles = {}

            def load_mask(qh, jj):
                mT = mb.tile([128, nq], BF16, tag="mt", name="mT")
                nc.sync.dma_start(
                    out=mT[:],
                    in_=maskT[jj * 128:(jj + 1) * 128, qh * nq:(qh + 1) * nq],
                )
                mask_tiles[(qh, jj)] = mT

            # ---- Phase A: projections (qT/kT packed over heads, v natural),
            # contraction chunk outermost so the PE streams as soon as the
            # first hidden chunk lands and never starves.  All loads are
            # straight bf16 DMAs (host pre-converted) ----
            with (
                tc.tile_pool(name="proj_ps", bufs=1, space="PSUM") as pp,
                tc.tile_pool(name="proj_sb", bufs=1) as psb,
            ):
                wq_sb = psb.tile([128, n_cc * 128], BF16)
                wk_sb = psb.tile([128, n_cc * 128], BF16)
                wv_sb = psb.tile([128, n_cc * 128], BF16)
                hT_cc = [
                    psb.tile([128, lq], BF16, name=f"hT{cc}") for cc in range(n_cc)
                ]
                for w_dram, w_bf in ((wq2, wq_sb), (wk2, wk_sb), (wv2, wv_sb)):
                    nc.sync.dma_start(
                        out=w_bf.rearrange("p (cc x) -> p cc x", x=128),
                        in_=w_dram.rearrange("(cc p) x -> p cc x", p=128),
                    )
                for cc in range(n_cc):
                    for qtr in range(4):
                        qs_ = slice(qtr * lq // 4, (qtr + 1) * lq // 4)
                        nc.sync.dma_start(
                            out=hT_cc[cc][:, qs_],
                            in_=hT[cc * 128:(cc + 1) * 128, qs_],
                        )

                # preload the ACT exp table while projections run
                warm = psb.tile([1, 1], F32)
                nc.vector.memset(warm[:], 0.0)
                nc.scalar.activation(
                    warm[:], warm[:], mybir.ActivationFunctionType.Exp
                )

                nc.sync.dma_start(out=kbgT_sb[:], in_=kbgT[:])
                for h in range(HPC):
                    nc.vector.memset(vbg_sb[h][:], 1.0)
                    # vbg shipped pre-scaled by ALPHA; land it straight in
                    # the strided [*, VE] slots around the ones column
                    nc.sync.dma_start(
                        out=vbg_sb[h].rearrange("p (t e) -> p t e", e=VE)[:, :, 0:DH],
                        in_=vbg[h].rearrange("(t p) d -> p t d", p=128),
                    )
                nc.sync.dma_start(out=wo_sb[:], in_=wo2[:])

                # projections, contraction-chunk outer
                pbw = min(lq, 512)
                nps = lq // pbw
                for w_sb, dstT in ((wq_sb, qT), (wk_sb, kT)):
                    pss = [
                        pp.tile([128, pbw], F32, tag=f"proj{nb}", name="ps")
                        for nb in range(nps)
                    ]
                    for cc in range(n_cc):
                        for nb in range(nps):
                            nc.tensor.matmul(
                                pss[nb][:],
                                lhsT=w_sb[:, cc * 128:(cc + 1) * 128],
                                rhs=hT_cc[cc][:, nb * pbw:(nb + 1) * pbw],
                                start=(cc == 0),
                                stop=(cc == n_cc - 1),
                            )
                    for nb in range(nps):
                        nc.vector.tensor_copy(
                            dstT[:, nb * pbw:(nb + 1) * pbw], pss[nb][:]
                        )
                for h in range(HPC):
                    nc.vector.memset(vself[h][:], 1.0)
                for tt in range(n_ts):
                    psv = pp.tile([128, HPC * DH], F32, tag="projv", name="psv", bufs=2)
                    for cc in range(n_cc):
                        nc.tensor.matmul(
                            psv[:],
                            lhsT=hT_cc[cc][:, tt * 128:(tt + 1) * 128],
                            rhs=wv_sb[:, cc * 128:(cc + 1) * 128],
                            start=(cc == 0),
                            stop=(cc == n_cc - 1),
                        )
                    for h in range(HPC):
                        nc.vector.tensor_copy(
                            vself[h][:, tt * VE: tt * VE + DH],
                            psv[:, h * DH:(h + 1) * DH],
                        )

            # ---- Phase B: attention; normalize + output projection of each
            # q block deferred into the next block's bg section ----
            with (
                tc.tile_pool(name="s_ps", bufs=2, space="PSUM") as sp,
                tc.tile_pool(name="c_ps", bufs=1, space="PSUM") as cp,
            ):

                def norm_cols(lo, hi):
                    # ctx2 = ctxr / den over columns [lo, hi).  The den row is
                    # spread across 128 partitions via a DRAM round-trip so the
                    # reciprocal costs w/128 DVE lanes-cycles instead of w;
                    # the reciprocal then broadcasts back over partitions
                    # (DMA from DRAM may broadcast, SBUF may not).
                    w = hi - lo
                    rb = ab.tile([128, w], F32, tag="rb", name="rb", bufs=2)
                    for h in range(HPC):
                        rdram = dp.tile([1, w], F32, tag="rd", name="rdram")
                        nc.sync.dma_start(out=rdram[:], in_=dens[h][:, lo:hi])
                        rsp = ab.tile([128, w // 128], F32, tag="rsp", name="rsp", bufs=2)
                        nc.sync.dma_start(
                            out=rsp[:],
                            in_=rdram.rearrange("o (p x) -> (o p) x", p=128),
                        )
                        rspr = ab.tile([128, w // 128], F32, tag="rspr", name="rspr", bufs=2)
                        nc.vector.reciprocal(rspr[:], rsp[:])
                        rdram2 = dp.tile([1, w], F32, tag="rd2", name="rdram2")
                        nc.sync.dma_start(
                            out=rdram2.rearrange("o (p x) -> (o p) x", p=128),
                            in_=rspr[:],
                        )
                        nc.sync.dma_start(
                            out=rb[h * DH:(h + 1) * DH, :],
                            in_=rdram2.to_broadcast((DH, w)),
                        )
                    nc.vector.tensor_tensor(
                        out=ctx2[:, lo:hi],
                        in0=ctxr[:, lo:hi],
                        in1=rb[:],
                        op=mybir.AluOpType.mult,
                    )

                def wo_cols(lo, hi):
                    # output projection over query rows [lo, hi) (PSUM slots
                    # shared with the scores pool)
                    for tt in range(lo // 128, hi // 128):
                        for nb in range(c // ncol):
                            po = sp.tile([128, ncol], F32, tag="s", name="po")
                            nc.tensor.matmul(
                                po[:],
                                lhsT=ctx2[:, tt * 128:(tt + 1) * 128],
                                rhs=wo_sb[:, nb * ncol:(nb + 1) * ncol],
                                start=True,
                                stop=True,
                            )
                            osb = ab.tile([128, ncol], F32, tag="ob", name="osb")
                            nc.vector.tensor_copy(osb[:], po[:])
                            nc.sync.dma_start(
                                out=outp[tt * 128:(tt + 1) * 128,
                                         nb * ncol:(nb + 1) * ncol],
                                in_=osb[:],
                            )

                for qh in range(n_qh):
                    qs = slice(qh * nq, (qh + 1) * nq)
                    Ch = [
                        cp.tile([DH + 1, nq], F32, tag=f"c{h}", name=f"ch{h}")
                        for h in range(HPC)
                    ]
                    # bg chunks first: they need no mask and no DVE work, so
                    # block boundaries never stall on mask DMA or the vector
                    # engine, and this block's masks prefetch during bg
                    order = list(range(n_ts, n_j)) + list(range(n_ts))
                    for oi, j in enumerate(order):
                        if oi == 0:
                            for jj2 in range(min(8, n_ts)):
                                load_mask(qh, jj2)
                        if oi == 4:
                            for jj2 in range(min(8, n_ts), n_ts):
                                load_mask(qh, jj2)
                        if oi == n_tb // 2 and qh > 0:
                            norm_cols((qh - 1) * nq, qh * nq)
                            wo_cols((qh - 1) * nq, qh * nq)
                        is_self = j < n_ts
                        jj = j if is_self else j - n_ts
                        if is_self:
                            mT = mask_tiles.pop((qh, jj))
                        # scores for both heads back-to-back: K=64 row-tiles
                        # (0,0)/(64,0) run concurrently on the PE
                        kTs = kT if is_self else kbgT_sb
                        S = [
                            sp.tile([128, nq], F32, tag="s", name=f"S{h}")
                            for h in range(HPC)
                        ]
                        for nb in range(nq // nw):
                            ns = slice(nb * nw, (nb + 1) * nw)
                            for h in range(HPC):
                                hs = slice(h * DH, (h + 1) * DH)
                                nc.tensor.matmul(
                                    S[h][:, ns],
                                    lhsT=kTs[hs, jj * 128:(jj + 1) * 128],
                                    rhs=qT[hs, qh * nq + nb * nw: qh * nq + (nb + 1) * nw],
                                    start=True, stop=True,
                                )
                        Ps = []
                        for h in range(HPC):
                            P = ab.tile([128, nq], BF16, tag="p", name="P", bufs=8)
                            if is_self:
                                Pe = ab.tile([128, nq], FP16, tag="pe", name="Pe", bufs=4)
                                nc.scalar.activation(
                                    Pe[:], S[h][:], mybir.ActivationFunctionType.Exp,
                                    scale=SCALE,
                                )
                                nc.vector.tensor_tensor(
                                    out=P[:], in0=Pe[:], in1=mT[:],
                                    op=mybir.AluOpType.mult,
                                )
                            else:
                                nc.scalar.activation(
                                    P[:], S[h][:], mybir.ActivationFunctionType.Exp,
                                    scale=ALPHA * SCALE,
                                )
                            Ps.append(P)
                        for h in range(HPC):
                            vext = (vself if is_self else vbg_sb)[h][
                                :, jj * VE:(jj + 1) * VE
                            ]
                            for nb in range(nq // nw):
                                ns = slice(nb * nw, (nb + 1) * nw)
                                nc.tensor.matmul(
                                    Ch[h][:, ns], lhsT=vext, rhs=Ps[h][:, ns],
                                    start=(oi == 0), stop=(oi == n_j - 1),
                                )
                    # drain the PSUM accumulators quickly so the next q block
                    # can reuse them; normalization happens a q block later
                    for h in range(HPC):
                        nc.vector.tensor_copy(dens[h][:, qs], Ch[h][DH:DH + 1, :])
                        nc.vector.tensor_copy(
                            ctxr[h * DH:(h + 1) * DH, qs], Ch[h][0:DH, :]
                        )
                # tail: last q block in pipelined column halves
                last = n_qh - 1
                for half in range(2):
                    lo = last * nq + half * (nq // 2)
                    hi = lo + nq // 2
                    norm_cols(lo, hi)
                    wo_cols(lo, hi)

    nc.compile()
    return nc


_NC_CACHE = {}


def _get_nc(key=(LQ, LBG, C)):
    if key not in _NC_CACHE:
        _NC_CACHE[key] = build_program(*key)
    return _NC_CACHE[key]


def make_in_maps(hidden_states, attention_mask, K_bg, V_bg, Wq, Wk, Wv, Wo):
    f = lambda a: np.ascontiguousarray(np.asarray(a, dtype=np.float32).astype(BF16_NP))
    hiddenT = [f(np.asarray(hidden_states)[b].T) for b in range(B)]
    maskT = [f(np.exp(np.asarray(attention_mask, dtype=np.float32)[b].T)) for b in range(B)]
    K_bg, V_bg = np.asarray(K_bg), np.asarray(V_bg)
    V_bg_s = V_bg * np.float32(ALPHA)
    Wq, Wk, Wv, Wo = map(np.asarray, (Wq, Wk, Wv, Wo))
    in_maps = []
    for core in range(N_CORES):
        bh0 = HPC * core
        b = bh0 // H
        h0 = bh0 % H
        cs = slice(h0 * DH, (h0 + HPC) * DH)
        in_maps.append({
            "hT": hiddenT[b],
            "maskT": maskT[b],
            "kbgT": f(K_bg[bh0:bh0 + HPC].transpose(0, 2, 1).reshape(HPC * DH, LBG)),
            "vbg": f(V_bg_s[bh0:bh0 + HPC]),
            "wq2": f(Wq[:, cs]),
            "wk2": f(Wk[:, cs]),
            "wv2": f(Wv[:, cs]),
            "wo2": f(Wo[cs, :]),
        })
    return in_maps


def _run(in_maps, trace=False, **kw):
    nc = _get_nc()
    return run_bass_kernel_spmd(nc, in_maps, list(range(N_CORES)), trace=trace, **kw)


def kernel(hidden_states, attention_mask, K_bg, V_bg, Wq, Wk, Wv, Wo, bo):
    in_maps = make_in_maps(
        hidden_states, attention_mask, K_bg, V_bg, Wq, Wk, Wv, Wo
    )
    res = _run(in_maps)
    out = np.zeros((B, LQ, C), np.float32)
    for core in range(N_CORES):
        out[core // (N_CORES // B)] += res.results[core]["outp"]
    out += np.asarray(bo, dtype=np.float32)
    return out
